# revision 1
# baseline (speedup 1.0000x reference)
"""AWQ int4 GEMM (M=1024, K=4096, N=11008, group_size=128) on 8 TRN2 NeuronCores.

Column-parallel tensor sharding (vLLM-style): qweight/qzeros/scales split
along N across the 8 cores, activations replicated, outputs concatenated.
Each core is fully independent (no collectives).

Host side (layout prep only, no FLOPs): slice the weight tensors per core,
cast/transpose the activation to the lhsT layout the TensorE consumes
(bf16 [K, M]), permute scale columns into the device's pair-block order and
replicate each scale row so the per-k-tile broadcast is one contiguous DMA.

Per-core device kernel (all dequant + GEMM compute), fully pipelined:
  - Phase A (kt = 0..31): per k-tile-pair, qweight int32 chunks are unpacked
    on the DVE with the "pair trick" ((q >> 4t) & 0x000F000F yields AWQ
    nibbles (2t, 2t+1) as packed int16 lanes; d-major nib layout so one
    2x-mode tensor_tensor dequantizes BOTH k-tiles of the pair against a
    [128, 2, N] scale broadcast).  Interleaved with the prep, the SAME kt
    loop runs m-tiles 0..1's PSUM accumulation plus the xsum one-hot
    matmuls for m-columns 0:512, so the PE computes the main GEMM while
    the DVE/DMA build W just ahead of it.  x is loaded column-half 0:512
    only (halves phase-A DMA).
  - zero-points are NOT subtracted elementwise:
        out = x @ (w4*s) - xsum_g @ (z_g * s_g)
    a rank-32 correction matmul appended to each m-tile's accumulation;
    xsum is computed on the TensorE with a one-hot stationary operand.
  - Phase B: m-tiles 2..7 run back-to-back on one PSUM pool with tag
    rotation (b0 x3 / b1 x2 / b2 x2 / ps1 x1 = exactly 8 banks) so there
    are no pool-scope barriers between m-tiles; each m-tile's correction +
    unpermute copies + output DMA drain while the next m-tile computes.
    x column-half 512:1024 streams in during m2's loop; the second xsum
    half rides m4's loop.
  - redundant LDWEIGHTS removed by a custom post-pass; pair-block
    permutation undone on the ScalarE during PSUM->SBUF output copies.
"""

import os
import sys
import types

sys.path.insert(0, "/opt/trn_rl_repo")

import numpy as np
import ml_dtypes

import concourse.bass as bass
import concourse.bass_isa as bass_isa
import concourse.mybir as mybir
import concourse.tile as tile
import bass_rust as _br
from concourse.vector_clock import ScopedClock
from concourse.bass_utils import run_bass_kernel_spmd


# ---------------------------------------------------------------------------
# Walrus workaround: this toolchain rejects >1 sem wait per instruction
# (2 for InstEventSemaphore).  Tile's sem assigner can emit more; split the
# excess onto no-ops placed immediately before on the same engine.
# ---------------------------------------------------------------------------
_orig_lower = tile.TileContext._lower_ordered_insts
_wsplit_counter = [0]


def _split_waits_in_place(nc, insts):
    new_list = []
    for inst in insts:
        si = inst.sync_info
        cap = 2 if isinstance(inst, mybir.InstEventSemaphore) else 1
        if si is not None and len(si.on_wait) > cap:
            waits = list(si.on_wait)
            extra, keep = waits[:-cap], waits[-cap:]
            for w in extra:
                _wsplit_counter[0] += 1
                nop = mybir.InstNoOp(
                    name=f"wsplit-{_wsplit_counter[0]}",
                    engine=inst.engine,
                    sync_info=mybir.SyncInfo(on_wait=[w], on_update=[]),
                    bass_nofuse=True,
                )
                nc.register_instruction(nop)
                new_list.append(nop)
            inst.sync_info = mybir.SyncInfo(on_wait=keep, on_update=list(si.on_update))
        new_list.append(inst)
    insts[:] = new_list


def _dedup_ldweights_in_place(nc, insts):
    """Drop InstLdweights identical to the still-loaded stationary operand.

    Weights stay resident in the PE array across matmuls, so an
    InstLdweights whose operand matches the previous one (with only
    matmuls in between) is redundant; its sync info moves to a no-op.
    """
    last_sig = None
    out = []
    for inst in insts:
        nm = inst.__class__.__name__
        if nm == "InstLdweights":
            sig = repr(inst.ins[0])
            if sig == last_sig:
                si = inst.sync_info
                if si is not None and (si.on_wait or si.on_update):
                    _wsplit_counter[0] += 1
                    nop = mybir.InstNoOp(
                        name=f"ldwkill-{_wsplit_counter[0]}",
                        engine=inst.engine,
                        sync_info=mybir.SyncInfo(
                            on_wait=list(si.on_wait),
                            on_update=list(si.on_update),
                        ),
                        bass_nofuse=True,
                    )
                    nc.register_instruction(nop)
                    out.append(nop)
                continue
            last_sig = sig
        elif nm != "InstMatmult" and inst.engine == mybir.EngineType.PE:
            last_sig = None
        out.append(inst)
    insts[:] = out


import os as _os
_DEDUP = _os.environ.get("AWQ_LDW_DEDUP", "1") == "1"
# GpSimd offloads (set to "0" to fall back to DMA/PE paths)
_GPS_BCAST = _os.environ.get("AWQ_GPS_BCAST", "1") == "1"
# partition_all_reduce works, but scattering its replicated result into
# xsumT rows hits the engines' quadrant-aligned partition-offset rule;
# PE one-hot fallback stays the default.
_GPS_XSUM = _os.environ.get("AWQ_GPS_XSUM", "0") == "1"


def _patched_lower(self, ordered):
    for insts in ordered.values():
        if _DEDUP:
            _dedup_ldweights_in_place(self.nc, insts)
        _split_waits_in_place(self.nc, insts)
    return _orig_lower(self, ordered)


def _patched_drain_and_barrier(self, tick_clock, wait_clock):
    nc = self.nc
    drain_inst = nc.sync.drain()
    wait_clock.add_sem_waits(
        drain_inst.ins, ScopedClock({None: tick_clock.global_clock})
    )
    si = drain_inst.ins.sync_info
    if si is not None and len(si.on_wait) > 1:
        waits = list(si.on_wait)
        drain_inst.ins.sync_info = _br.SyncInfo(
            on_wait=[waits[0]], on_update=list(si.on_update)
        )
        for w in waits[1:]:
            extra = nc.sync.drain()
            extra.ins.sync_info = _br.SyncInfo(on_wait=[w], on_update=[])
    nc.all_engine_barrier()
    assert self.sems is not None
    popped = nc._tile_sem_poison_stack.pop()
    assert popped is self._sem_poison
    nc.clear_and_free_semaphores(list(self.sems.allocated().values()))
    nc.all_engine_barrier()


tile.TileContext._lower_ordered_insts = _patched_lower
tile.TileContext._drain_and_barrier = _patched_drain_and_barrier

# ---------------------------------------------------------------------------
# NTFF profiling hook shim (the agent image's antenv lacks axon_hooks; the
# hook machinery itself is present in trn_agent_boot).  Only used when
# tracing is requested.
# ---------------------------------------------------------------------------
def _install_ntff_shim():
    if "antenv.axon_hooks" in sys.modules:
        return
    try:
        from trn_agent_boot.trn_boot import _ntff_profile_via_ctypes

        hook = _ntff_profile_via_ctypes("/opt/axon/libaxon_pjrt.so")
    except Exception:
        hook = None
    m = types.ModuleType("antenv.axon_hooks")
    m.get_axon_ntff_profile_hook = lambda: hook
    m.set_axon_ntff_profile_hook = lambda h: None
    import antenv  # noqa: F401

    sys.modules["antenv.axon_hooks"] = m


# ---------------------------------------------------------------------------
# Problem shape (hardcoded per contract)
# ---------------------------------------------------------------------------
M, K, N_TOTAL = 1024, 4096, 11008
NCORES = 8
N_LOC = N_TOTAL // NCORES  # 1376 unpacked columns per core
NP = N_LOC // 8            # 172 packed int32 columns per core
G = 32                     # scale/zero groups (group_size 128 == k-tile)
KT = K // 128              # 32 k-tiles
MT = M // 128              # 8 m-tiles
NB = 4                     # pair-blocks per core (one per unpack shift)
BW = N_LOC // NB           # 344 columns per pair-block

PAIR_SHIFTS = (0, 4, 8, 12)
SREP = 128  # scale-row replication factor in DRAM
PAIR_MASK = 0x000F000F
QC = 2      # k-tiles per qweight prefetch chunk

F32 = mybir.dt.float32
BF16 = mybir.dt.bfloat16
I32 = mybir.dt.int32
I16 = mybir.dt.int16

LAST_EXEC_NS = None
LAST_TRACE = None

_cached_nc = None


def _build():
    nc = bass.Bass()
    xt_d = nc.declare_dram_parameter("xt", [K, M], BF16, isOutput=False)
    # qweight pre-swizzled on host to partition-major [128, KT, NP] so a
    # prefetch chunk is one contiguous 1376B run per partition.
    qw_d = nc.declare_dram_parameter("qw", [128, KT, NP], I32, isOutput=False)
    # scales replica-major [SREP, G, N_LOC]: partition p reads rows kt,kt+1
    # of replica p as one contiguous 5504B descriptor.
    sp_d = nc.declare_dram_parameter("sp", [SREP, G, N_LOC], BF16, isOutput=False)
    qz_d = nc.declare_dram_parameter("qz", [G, NP], I32, isOutput=False)
    out_d = nc.declare_dram_parameter("out", [M, N_LOC], BF16, isOutput=True)

    AND = mybir.AluOpType.bitwise_and
    LSR = mybir.AluOpType.logical_shift_right
    MUL = mybir.AluOpType.mult

    # psum block boundaries and their pair-block (t) decomposition
    PSB = [(0, 512), (512, 1024), (1024, N_LOC)]

    def unperm_pieces(lo, hi):
        pieces = []
        pos = lo
        while pos < hi:
            t = pos // BW
            nxt = min(hi, (t + 1) * BW)
            pieces.append((pos - lo, nxt - lo, t, pos - t * BW))
            pos = nxt
        return pieces

    with tile.TileContext(nc) as tc:
        from contextlib import ExitStack

        with ExitStack() as ctx:
            big = ctx.enter_context(tc.tile_pool(name="big", bufs=1))
            xT = big.tile([128, KT, M], BF16)       # x transposed (k on part)
            W = big.tile([128, KT, N_LOC], BF16)    # dequant (w4*s), perm order

            consts = ctx.enter_context(tc.tile_pool(name="consts", bufs=1))
            onehot = consts.tile([128, 63], BF16)
            nc.vector.memset(onehot, 0.0)
            nc.vector.memset(onehot[:, 31:32], 1.0)

            sp_sb = consts.tile([G, N_LOC], BF16)
            nc.scalar.dma_start(out=sp_sb, in_=sp_d[0])
            qz_sb = consts.tile([G, NP], I32)
            nc.scalar.dma_start(out=qz_sb, in_=qz_d[:, :])
            znib = consts.tile([G, NB, NP], I32)
            ztmp = consts.tile([G, N_LOC], BF16)
            B_bf = consts.tile([G, N_LOC], BF16)   # -(z*s), perm order
            xsumT = consts.tile([G, M], BF16)      # per-group column sums of x

            wprep = ctx.enter_context(tc.tile_pool(name="wprep", bufs=2))
            opool = ctx.enter_context(tc.tile_pool(name="oout", bufs=2))

            def zb_prep():
                for t in range(NB):
                    nc.vector.tensor_scalar(
                        out=znib[:, t, :], in0=qz_sb,
                        scalar1=PAIR_SHIFTS[t], scalar2=PAIR_MASK,
                        op0=LSR, op1=AND,
                    )
                z16 = znib.bitcast(I16).rearrange("p a b -> p (a b)")
                nc.vector.tensor_tensor(out=ztmp, in0=z16, in1=sp_sb, op=MUL)
                nc.vector.tensor_scalar_mul(B_bf, ztmp, -1.0)

            def q_prefetch(chunk):
                qbuf = wprep.tile([128, QC, NP], I32, name="qbuf", tag="qbuf",
                                  bufs=2)
                nc.scalar.dma_start(
                    out=qbuf, in_=qw_d[:, chunk * QC:(chunk + 1) * QC, :]
                )
                return qbuf

            def w_prep_pair(kt, qbuf):
                # unpack+scale k-tiles kt, kt+1 with one fused unpack pass
                # per pair-block and ONE 2x-mode multiply for the pair.
                # nib is d-major: [128, d, t, NP] so nib16[:, d] is the
                # contiguous perm-order row for k-tile kt+d.
                nib = wprep.tile([128, 2, NB, NP], I32, name="nib", tag="nib",
                                 bufs=2)
                qs = qbuf[:, kt % QC:kt % QC + 2, :]
                for t in range(NB):
                    nc.vector.tensor_scalar(
                        out=nib[:, :, t, :], in0=qs,
                        scalar1=PAIR_SHIFTS[t], scalar2=PAIR_MASK,
                        op0=LSR, op1=AND,
                    )
                # scale rows kt, kt+1 broadcast to 128 partitions (each
                # partition reads its own replica from the host-replicated
                # DRAM copy).  The broadcast stream is the phase-A bandwidth
                # hog and a single descriptor ring caps well below the DMA
                # engines' aggregate throughput, so round-robin the pairs
                # over three independent descriptor rings: the two hardware
                # DGE rings (ScalarE/SP) plus the GpSimd software DGE.
                s2 = wprep.tile([128, 2, N_LOC], BF16, name="s2", tag="sbc",
                                bufs=3)
                pair = kt // 2
                if _GPS_BCAST:
                    eng = (nc.scalar, nc.sync, nc.gpsimd)[pair % 3]
                else:
                    eng = (nc.scalar, nc.sync)[pair % 2]
                rep = sp_d[0]  # [G, N_LOC] view of replica 0
                eng.dma_start(
                    out=s2,
                    in_=bass.AP(
                        tensor=rep.tensor,
                        offset=rep.offset + kt * N_LOC,
                        ap=[[G * N_LOC, 128], [N_LOC, 2], [1, N_LOC]],
                    ),
                )
                nib16 = nib.bitcast(I16)  # [128, 2, NB, 2*NP]
                nc.vector.tensor_tensor(
                    out=W[:, kt:kt + 2, :],
                    in0=nib16.rearrange("p d a b -> p d (a b)"),
                    in1=s2,
                    op=MUL,
                )

            def xa_load(kt):
                nc.sync.dma_start(
                    out=xT[:, kt, 0:512],
                    in_=xt_d[kt * 128:(kt + 1) * 128, 0:512],
                )

            def xb_load(kt):
                eng = nc.scalar if kt % 2 == 0 else nc.sync
                eng.dma_start(
                    out=xT[:, kt, 512:1024],
                    in_=xt_d[kt * 128:(kt + 1) * 128, 512:1024],
                )

            pb = ctx.enter_context(
                tc.tile_pool(name="pb", bufs=1, space="PSUM")
            )

            def mk_ps(m):
                # b0 gets 3 bufs so m+2 can start immediately while m drains
                return [
                    pb.tile([128, hi - lo], F32, name=f"ps_{m}_{i}",
                            tag=f"b{i}", bufs=3 if i == 0 else 2)
                    for i, (lo, hi) in enumerate(PSB)
                ]

            def drain_m(m, ps):
                for i, (lo, hi) in enumerate(PSB):
                    nc.tensor.matmul(
                        ps[i],
                        lhsT=xsumT[:, m * 128:(m + 1) * 128],
                        rhs=B_bf[:, lo:hi],
                        start=False, stop=True,
                        skip_group_check=True,
                    )
                out_sb = opool.tile([128, N_LOC], BF16, name="osb", tag="osb")
                o3 = out_sb.rearrange("p (c j) -> p c j", j=8)
                for i, (lo, hi) in enumerate(PSB):
                    pv = ps[i].rearrange("p (c r) -> p c r", r=2)
                    for (llo, lhi, t, ilo) in unperm_pieces(lo, hi):
                        nc.scalar.copy(
                            o3[:, ilo // 2:(ilo + lhi - llo) // 2,
                               2 * t:2 * t + 2],
                            pv[:, llo // 2:lhi // 2, :],
                        )
                nc.sync.dma_start(
                    out=out_d[m * 128:(m + 1) * 128, :], in_=out_sb
                )

            # ---- phase A: prep pipeline + m0/m1 GEMM + xsum(0:512) ----
            ps_m0 = mk_ps(0)
            ps_m1 = mk_ps(1)
            psx_a = pb.tile([G, 512], F32, name="psx_a", tag="ps1", bufs=1)
            qbuf = None
            for kt in range(KT):
                if kt % QC == 0:
                    qbuf = q_prefetch(kt // QC)
                if kt % 2 == 0:
                    w_prep_pair(kt, qbuf)
                xa_load(kt)
                if kt == 2:
                    zb_prep()
                nc.tensor.matmul(
                    psx_a,
                    lhsT=onehot[:, 31 - kt:63 - kt],
                    rhs=xT[:, kt, 0:512],
                    start=(kt == 0), stop=(kt == KT - 1),
                    skip_group_check=True,
                )
                for m, ps in ((0, ps_m0), (1, ps_m1)):
                    for i, (lo, hi) in enumerate(PSB):
                        nc.tensor.matmul(
                            ps[i],
                            lhsT=xT[:, kt, m * 128:(m + 1) * 128],
                            rhs=W[:, kt, lo:hi],
                            start=(kt == 0), stop=False,
                            skip_group_check=True,
                        )
            nc.vector.tensor_copy(xsumT[:, 0:512], psx_a)
            drain_m(0, ps_m0)
            drain_m(1, ps_m1)

            # x column-half 512:1024.  These have no data dependencies, so
            # the tile scheduler would hoist them into phase A where they
            # compete with the scale broadcasts for DMA bandwidth; pin them
            # past phase A on the scheduler's modeled timeline instead.
            # (Runtime sems still guarantee correctness if the model drifts.)
            with tc.tile_wait_until(0.05):
                for kt in range(KT):
                    xb_load(kt)

            # ---- phase B: m2..m7; xsum(512:1024) on the GpSimd (partition
            # all-reduce per k-tile during m2's window, one ScalarE row-copy
            # into xsumT) or, as fallback, one-hot matmuls riding m4's loop.
            for m in range(2, 8):
                ps = mk_ps(m)
                psx_b = (
                    pb.tile([G, 512], F32, name="psx_b", tag="ps1", bufs=1)
                    if (m == 4 and not _GPS_XSUM) else None
                )
                for kt in range(KT):
                    if m == 2 and _GPS_XSUM:
                        scr = wprep.tile([128, 512], F32, name="xscr",
                                         tag="xscr", bufs=2)
                        nc.gpsimd.partition_all_reduce(
                            scr, xT[:, kt, 512:1024],
                            channels=128,
                            reduce_op=bass_isa.ReduceOp.add,
                        )
                        nc.scalar.copy(
                            xsumT[kt:kt + 1, 512:1024], scr[kt:kt + 1, :]
                        )
                    if m == 4 and not _GPS_XSUM:
                        nc.tensor.matmul(
                            psx_b,
                            lhsT=onehot[:, 31 - kt:63 - kt],
                            rhs=xT[:, kt, 512:1024],
                            start=(kt == 0), stop=(kt == KT - 1),
                            skip_group_check=True,
                        )
                    for i, (lo, hi) in enumerate(PSB):
                        nc.tensor.matmul(
                            ps[i],
                            lhsT=xT[:, kt, m * 128:(m + 1) * 128],
                            rhs=W[:, kt, lo:hi],
                            start=(kt == 0), stop=False,
                            skip_group_check=True,
                        )
                if m == 4 and not _GPS_XSUM:
                    nc.vector.tensor_copy(xsumT[:, 512:1024], psx_b)
                drain_m(m, ps)

    return nc


def _get_nc():
    global _cached_nc
    if _cached_nc is None:
        _cached_nc = _build()
    return _cached_nc


def kernel(x, qweight, scales, qzeros):
    global LAST_EXEC_NS, LAST_TRACE

    x = np.asarray(x, dtype=np.float32)
    x_t = np.ascontiguousarray(x.T.astype(ml_dtypes.bfloat16))
    qweight = np.asarray(qweight, dtype=np.int32)
    scales = np.asarray(scales, dtype=np.float32)
    qzeros = np.asarray(qzeros, dtype=np.int32)

    in_maps = []
    for c in range(NCORES):
        # partition-major qweight: qw_pm[p, a, :] = qweight[a*128 + p, cols]
        qw_c = qweight[:, c * NP:(c + 1) * NP]
        qw_pm = np.ascontiguousarray(
            qw_c.reshape(KT, 128, NP).transpose(1, 0, 2)
        )
        qz_c = np.ascontiguousarray(qzeros[:, c * NP:(c + 1) * NP])
        s_c = scales[:, c * N_LOC:(c + 1) * N_LOC]
        # pair-block permutation: dest[g, 344*t + 2*cc + r] = s[g, 8*cc + 2*t + r]
        s_perm = np.ascontiguousarray(
            s_c.reshape(G, NP, 4, 2).transpose(0, 2, 1, 3).reshape(G, N_LOC)
        ).astype(ml_dtypes.bfloat16)
        # replica-major: [SREP, G, N_LOC]
        s_rep = np.ascontiguousarray(
            np.broadcast_to(s_perm[None, :, :], (SREP, G, N_LOC))
        )
        in_maps.append({"xt": x_t, "qw": qw_pm, "sp": s_rep, "qz": qz_c})

    trace = os.environ.get("AWQ_KERNEL_TRACE", "0") == "1"
    if trace:
        _install_ntff_shim()

    nc = _get_nc()
    res = run_bass_kernel_spmd(
        nc, in_maps, core_ids=list(range(NCORES)), trace=trace
    )
    LAST_EXEC_NS = res.exec_time_ns
    if res.instructions_and_trace is not None:
        LAST_TRACE = res.instructions_and_trace[1]

    return np.concatenate(
        [np.asarray(res.results[i]["out"]).astype(np.float32)
         for i in range(NCORES)],
        axis=1,
    )



# revision 18
# speedup vs baseline: 1.0524x; 1.0524x over previous
"""AWQ int4 GEMM (M=1024, K=4096, N=11008, group_size=128) on 8 TRN2 NeuronCores.

Column-parallel tensor sharding (vLLM-style): qweight/qzeros/scales split
along N across the 8 cores, activations replicated, outputs concatenated.
Each core is fully independent (no collectives).

v2 structure — three N-phases so the PE never waits on dequant DMA:
  The dequantized weight tile W_ext [128, KT, 1408] stays fully resident in
  SBUF, and the PE sweeps it in three column passes, each a plain
  kt(outer) x m(inner) accumulation into 8 PSUM banks (one per m-tile):
    phase 0: cols    0: 512  = perm-cols 0:480  ++ 32 one-hot columns
    phase 1: cols  512:1024  = perm-cols 480:992
    phase 2: cols 1024:1408  = perm-cols 992:1376
  The scale broadcast (the phase-A bandwidth hog in v1: 11.3 MB of
  128x-replicated rows) is split into per-phase column slices, so its DMA
  spreads over the whole kernel instead of one window; dequant for phase
  p+1 runs while the PE consumes phase p (the W tile gives ~55us of
  prefetch slack per phase).

  The one-hot columns compute xsum on the PE for free: column 480+g of
  k-tile kt's rhs is ones iff g==kt, so after phase 0 psum cols 480:512
  hold xsum[m-row, g] = sum_{k in group g} x[m, k].  That replaces v1's 64
  dedicated one-hot matmuls (21.8us of PE busy).  Per m-tile the [128,32]
  xsum block is cast to bf16, transposed by the DMA XBAR
  (dma_start(transpose=True), 8 tiles x 14ns), and used as the stationary
  operand of the per-phase zero-point correction matmul
  out -= xsum_g @ (z_g * s_g).

  Pair-trick dequant unchanged from v1: (q >> 4t) & 0x000F000F yields AWQ
  nibbles (2t, 2t+1) as packed int16 lanes; one tensor_tensor multiplies
  both k-tiles of a pair against the per-partition-replicated scale rows.
  Per phase only the packed-column pieces feeding that phase's perm-column
  range are unpacked.
"""

import os
import sys
import types

sys.path.insert(0, "/opt/trn_rl_repo")

import numpy as np
import ml_dtypes

import concourse.bass as bass
import concourse.bass_isa as bass_isa
import concourse.mybir as mybir
import concourse.tile as tile
import bass_rust as _br
from concourse.vector_clock import ScopedClock
from concourse.bass_utils import run_bass_kernel_spmd


# ---------------------------------------------------------------------------
# Walrus workaround: this toolchain rejects >1 sem wait per instruction
# (2 for InstEventSemaphore).  Tile's sem assigner can emit more; split the
# excess onto no-ops placed immediately before on the same engine.
# ---------------------------------------------------------------------------
_orig_lower = tile.TileContext._lower_ordered_insts
_wsplit_counter = [0]


def _split_waits_in_place(nc, insts):
    new_list = []
    for inst in insts:
        si = inst.sync_info
        cap = 2 if isinstance(inst, mybir.InstEventSemaphore) else 1
        if si is not None and len(si.on_wait) > cap:
            waits = list(si.on_wait)
            extra, keep = waits[:-cap], waits[-cap:]
            for w in extra:
                _wsplit_counter[0] += 1
                nop = mybir.InstNoOp(
                    name=f"wsplit-{_wsplit_counter[0]}",
                    engine=inst.engine,
                    sync_info=mybir.SyncInfo(on_wait=[w], on_update=[]),
                    bass_nofuse=True,
                )
                nc.register_instruction(nop)
                new_list.append(nop)
            inst.sync_info = mybir.SyncInfo(on_wait=keep, on_update=list(si.on_update))
        new_list.append(inst)
    insts[:] = new_list


def _dedup_ldweights_in_place(nc, insts):
    """Drop InstLdweights identical to the still-loaded stationary operand."""
    last_sig = None
    out = []
    for inst in insts:
        nm = inst.__class__.__name__
        if nm == "InstLdweights":
            sig = repr(inst.ins[0])
            if sig == last_sig:
                si = inst.sync_info
                if si is not None and (si.on_wait or si.on_update):
                    _wsplit_counter[0] += 1
                    nop = mybir.InstNoOp(
                        name=f"ldwkill-{_wsplit_counter[0]}",
                        engine=inst.engine,
                        sync_info=mybir.SyncInfo(
                            on_wait=list(si.on_wait),
                            on_update=list(si.on_update),
                        ),
                        bass_nofuse=True,
                    )
                    nc.register_instruction(nop)
                    out.append(nop)
                continue
            last_sig = sig
        elif nm != "InstMatmult" and inst.engine == mybir.EngineType.PE:
            last_sig = None
        out.append(inst)
    insts[:] = out


def _patched_lower(self, ordered):
    for insts in ordered.values():
        _dedup_ldweights_in_place(self.nc, insts)
        _split_waits_in_place(self.nc, insts)
    return _orig_lower(self, ordered)


def _patched_drain_and_barrier(self, tick_clock, wait_clock):
    nc = self.nc
    drain_inst = nc.sync.drain()
    wait_clock.add_sem_waits(
        drain_inst.ins, ScopedClock({None: tick_clock.global_clock})
    )
    si = drain_inst.ins.sync_info
    if si is not None and len(si.on_wait) > 1:
        waits = list(si.on_wait)
        drain_inst.ins.sync_info = _br.SyncInfo(
            on_wait=[waits[0]], on_update=list(si.on_update)
        )
        for w in waits[1:]:
            extra = nc.sync.drain()
            extra.ins.sync_info = _br.SyncInfo(on_wait=[w], on_update=[])
    nc.all_engine_barrier()
    assert self.sems is not None
    popped = nc._tile_sem_poison_stack.pop()
    assert popped is self._sem_poison
    nc.clear_and_free_semaphores(list(self.sems.allocated().values()))
    nc.all_engine_barrier()


tile.TileContext._lower_ordered_insts = _patched_lower
tile.TileContext._drain_and_barrier = _patched_drain_and_barrier

# ---------------------------------------------------------------------------
# NTFF profiling hook shim (only used when tracing is requested).
# ---------------------------------------------------------------------------
def _install_ntff_shim():
    if "antenv.axon_hooks" in sys.modules:
        return
    try:
        from trn_agent_boot.trn_boot import _ntff_profile_via_ctypes

        hook = _ntff_profile_via_ctypes("/opt/axon/libaxon_pjrt.so")
    except Exception:
        hook = None
    m = types.ModuleType("antenv.axon_hooks")
    m.get_axon_ntff_profile_hook = lambda: hook
    m.set_axon_ntff_profile_hook = lambda h: None
    import antenv  # noqa: F401

    sys.modules["antenv.axon_hooks"] = m


# ---------------------------------------------------------------------------
# Problem shape (hardcoded per contract)
# ---------------------------------------------------------------------------
M, K, N_TOTAL = 1024, 4096, 11008
NCORES = 8
N_LOC = N_TOTAL // NCORES  # 1376 unpacked columns per core
NP = N_LOC // 8            # 172 packed int32 columns per core
G = 32                     # scale/zero groups (group_size 128 == k-tile)
KT = K // 128              # 32 k-tiles
MT = M // 128              # 8 m-tiles
NB = 4                     # pair-blocks per core (one per unpack shift)
BW = N_LOC // NB           # 344 perm columns per pair-block

PAIR_SHIFTS = (0, 4, 8, 12)
SREP = 128  # scale-row replication factor in DRAM
PAIR_MASK = 0x000F000F

F32 = mybir.dt.float32
BF16 = mybir.dt.bfloat16
I32 = mybir.dt.int32
I16 = mybir.dt.int16

# Column phases: (perm_lo, perm_hi, W_ext col offset, psum width)
NEXT = N_LOC + 32          # 1408 = W_ext row width
PH = (
    (0, 480, 0, 512),      # + one-hot cols 480:512 of W_ext
    (480, 992, 512, 512),
    (992, 1376, 1024, 384),
)
# packed-column pieces (t, c0, c1) per phase: perm cols [344t+2c0, 344t+2c1)
PH_PIECES = (
    ((0, 0, 172), (1, 0, 68)),
    ((1, 68, 172), (2, 0, 152)),
    ((2, 152, 172), (3, 0, 172)),
)
# modeled-timeline pins (ms) for phase 1/2 dequant prefetch
PH_PIN = (None, 0.025, 0.065)

# debug switches
_NO_CORR = os.environ.get("AWQ_NO_CORR", "0") == "1"   # skip zero-point corr
_NO_XBAR = os.environ.get("AWQ_NO_XBAR", "0") == "1"   # skip xsum transpose
_DBG_XS = os.environ.get("AWQ_DBG_XS", "0") == "1"     # dump xsp/xsT

LAST_EXEC_NS = None
LAST_TRACE = None

_cached_nc = None


def _build():
    nc = bass.Bass()
    xt_d = nc.declare_dram_parameter("xt", [K, M], BF16, isOutput=False)
    # qweight pre-swizzled on host to partition-major [128, KT, NP] so a
    # pair chunk is one contiguous 1376B run per partition.
    qw_d = nc.declare_dram_parameter("qw", [128, KT, NP], I32, isOutput=False)
    # scales replica-major [SREP, G, N_LOC] (perm order): partition p reads
    # its own replica, per-phase column slices.
    sp_d = nc.declare_dram_parameter("sp", [SREP, G, N_LOC], BF16, isOutput=False)
    qz_d = nc.declare_dram_parameter("qz", [G, NP], I32, isOutput=False)
    oh_d = nc.declare_dram_parameter("oh", [128, G, G], BF16, isOutput=False)
    out_d = nc.declare_dram_parameter("out", [M, N_LOC], BF16, isOutput=True)


    AND = mybir.AluOpType.bitwise_and
    LSR = mybir.AluOpType.logical_shift_right
    MUL = mybir.AluOpType.mult

    def unperm_pieces(lo, hi):
        pieces = []
        pos = lo
        while pos < hi:
            t = pos // BW
            nxt = min(hi, (t + 1) * BW)
            pieces.append((pos - lo, nxt - lo, t, pos - t * BW))
            pos = nxt
        return pieces

    with tile.TileContext(nc) as tc:
        from contextlib import ExitStack

        with ExitStack() as ctx:
            big = ctx.enter_context(tc.tile_pool(name="big", bufs=1))
            xT = big.tile([128, KT, M], BF16)        # x transposed (k on part)
            W = big.tile([128, KT, NEXT], BF16)      # dequant + one-hot cols
            out_sb = big.tile([128, MT, N_LOC], BF16)

            consts = ctx.enter_context(tc.tile_pool(name="consts", bufs=1))
            sp_sb = consts.tile([G, N_LOC], BF16)
            qz_sb = consts.tile([G, NP], I32)
            znib = consts.tile([G, NB, NP], I32)
            ztmp = consts.tile([G, N_LOC], BF16)
            B_bf = consts.tile([G, N_LOC], BF16)     # -(z*s), perm order
            xsp = consts.tile([128, MT, 128], BF16)  # xsum staging (padded)
            xsT = consts.tile([128, MT, 128], BF16)  # transposed xsum per m

            wprep = ctx.enter_context(tc.tile_pool(name="wprep", bufs=2))

            # ---- priming loads: first dequant unit's inputs go first on
            # the scalar ring so the PE can start ASAP.
            def q_prefetch(ph, j):
                qbuf = wprep.tile([128, 2, NP], I32, name="qbuf", tag="qbuf",
                                  bufs=2)
                nc.scalar.dma_start(
                    out=qbuf, in_=qw_d[:, 2 * j:2 * j + 2, :]
                )
                return qbuf

            def dequant_pair(ph, j, qbuf):
                plo, phi, wlo, _ = PH[ph]
                pieces = PH_PIECES[ph]
                pw = sum(c1 - c0 for _, c0, c1 in pieces)
                kt = 2 * j
                nib = wprep.tile([128, 2, 256], I32, name="nib", tag="nib",
                                 bufs=2)
                off = 0
                for t, c0, c1 in pieces:
                    nc.vector.tensor_scalar(
                        out=nib[:, :, off:off + (c1 - c0)],
                        in0=qbuf[:, :, c0:c1],
                        scalar1=PAIR_SHIFTS[t], scalar2=PAIR_MASK,
                        op0=LSR, op1=AND,
                    )
                    off += c1 - c0
                # scale rows kt, kt+1 cols [plo, phi), one replica per
                # partition (2 segments of (phi-plo)*2B per partition)
                s2 = wprep.tile([128, 2, 2 * pw], BF16, name="s2", tag="sbc",
                                bufs=3)
                rep = sp_d[0]  # [G, N_LOC] view of replica 0
                nc.scalar.dma_start(
                    out=s2,
                    in_=bass.AP(
                        tensor=rep.tensor,
                        offset=rep.offset + kt * N_LOC + plo,
                        ap=[[G * N_LOC, 128], [N_LOC, 2], [1, phi - plo]],
                    ),
                )
                nib16 = nib.bitcast(I16)  # [128, 2, 512]
                nc.vector.tensor_tensor(
                    out=W[:, kt:kt + 2, wlo:wlo + 2 * pw],
                    in0=nib16[:, :, 0:2 * pw],
                    in1=s2,
                    op=MUL,
                )

            def zb_prep():
                for t in range(NB):
                    nc.vector.tensor_scalar(
                        out=znib[:, t, :], in0=qz_sb,
                        scalar1=PAIR_SHIFTS[t], scalar2=PAIR_MASK,
                        op0=LSR, op1=AND,
                    )
                z16 = znib.bitcast(I16).rearrange("p a b -> p (a b)")
                nc.vector.tensor_tensor(out=ztmp, in0=z16, in1=sp_sb, op=MUL)
                nc.vector.tensor_scalar_mul(B_bf, ztmp, -1.0)

            def x_load(kt):
                nc.sync.dma_start(
                    out=xT[:, kt, :],
                    in_=xt_d[kt * 128:(kt + 1) * 128, :],
                )

            # one-hot columns: W[p, kt, 480+g] = (g == kt), host-replicated
            nc.sync.dma_start(out=W[:, :, 480:512], in_=oh_d[:, :, :])
            nc.vector.memset(xsp, 0.0)

            pb = ctx.enter_context(
                tc.tile_pool(name="pb", bufs=1, space="PSUM")
            )

            nc.scalar.dma_start(out=sp_sb, in_=sp_d[0])
            nc.scalar.dma_start(out=qz_sb, in_=qz_d[:, :])
            zb_prep()

            def drain_m(ph, m, ps):
                plo, phi, _, _ = PH[ph]
                width = phi - plo
                if ph == 0:
                    # xsum block -> bf16 staging -> DMA XBAR transpose
                    nc.scalar.copy(xsp[:, m, 0:G], ps[:, 480:512])
                    if not _NO_XBAR:
                        nc.sync.dma_start(
                            out=xsT[:, m, :], in_=xsp[:, m, :], transpose=True
                        )
                # zero-point correction for this phase's columns
                if not _NO_CORR:
                    nc.tensor.matmul(
                        ps[:, 0:width],
                        lhsT=xsT[0:G, m, :],
                        rhs=B_bf[:, plo:phi],
                        start=False, stop=True,
                        skip_group_check=True,
                    )
                # un-permute copies into the staging tile
                o3 = out_sb[:, m, :].rearrange("p (c j) -> p c j", j=8)
                pv = ps[:, 0:width].rearrange("p (c r) -> p c r", r=2)
                for (llo, lhi, t, ilo) in unperm_pieces(plo, phi):
                    nc.scalar.copy(
                        o3[:, ilo // 2:(ilo + lhi - llo) // 2,
                           2 * t:2 * t + 2],
                        pv[:, llo // 2:lhi // 2, :],
                    )
                if ph == 2 and not _DBG_XS:
                    nc.sync.dma_start(
                        out=out_d[m * 128:(m + 1) * 128, :],
                        in_=out_sb[:, m, :],
                    )

            for ph in range(3):
                wlo, wwidth = PH[ph][2], PH[ph][3]
                pin = PH_PIN[ph]
                if pin is not None:
                    with tc.tile_wait_until(pin):
                        for j in range(KT // 2):
                            qbuf = q_prefetch(ph, j)
                            dequant_pair(ph, j, qbuf)
                else:
                    pass  # phase 0 dequant is interleaved with the kt loop
                ps = [
                    pb.tile([128, 512], F32, name=f"ps_{ph}_{m}",
                            tag=f"ps{m}", bufs=1)
                    for m in range(MT)
                ]
                for kt in range(KT):
                    if ph == 0:
                        if kt % 2 == 0:
                            qbuf = q_prefetch(0, kt // 2)
                            dequant_pair(0, kt // 2, qbuf)
                        x_load(kt)
                    for m in range(MT):
                        nc.tensor.matmul(
                            ps[m][:, 0:wwidth],
                            lhsT=xT[:, kt, m * 128:(m + 1) * 128],
                            rhs=W[:, kt, wlo:wlo + wwidth],
                            start=(kt == 0), stop=False,
                            skip_group_check=True,
                        )
                for m in range(MT):
                    drain_m(ph, m, ps[m])
                if ph == 0 and _DBG_XS:
                    # overwrite output rows 0:128 with xsp, 128:256 with xsT,
                    # 256:384 with the W one-hot region (debug only)
                    with tc.tile_wait_until(0.2):
                        nc.sync.dma_start(
                            out=out_d[0:128, 0:MT * 128],
                            in_=xsp.rearrange("p a b -> p (a b)"),
                        )
                        nc.sync.dma_start(
                            out=out_d[128:256, 0:MT * 128],
                            in_=xsT.rearrange("p a b -> p (a b)"),
                        )
                        nc.sync.dma_start(
                            out=out_d[256:384, 0:KT * G],
                            in_=W[:, :, 480:512],
                        )

    return nc


def _get_nc():
    global _cached_nc
    if _cached_nc is None:
        _cached_nc = _build()
    return _cached_nc


def kernel(x, qweight, scales, qzeros):
    global LAST_EXEC_NS, LAST_TRACE

    x = np.asarray(x, dtype=np.float32)
    x_t = np.ascontiguousarray(x.T.astype(ml_dtypes.bfloat16))
    qweight = np.asarray(qweight, dtype=np.int32)
    scales = np.asarray(scales, dtype=np.float32)
    qzeros = np.asarray(qzeros, dtype=np.int32)
    oh = np.ascontiguousarray(
        np.broadcast_to(np.eye(G, dtype=ml_dtypes.bfloat16)[None], (128, G, G))
    )

    in_maps = []
    for c in range(NCORES):
        # partition-major qweight: qw_pm[p, a, :] = qweight[a*128 + p, cols]
        qw_c = qweight[:, c * NP:(c + 1) * NP]
        qw_pm = np.ascontiguousarray(
            qw_c.reshape(KT, 128, NP).transpose(1, 0, 2)
        )
        qz_c = np.ascontiguousarray(qzeros[:, c * NP:(c + 1) * NP])
        s_c = scales[:, c * N_LOC:(c + 1) * N_LOC]
        # pair-block permutation: dest[g, 344*t + 2*cc + r] = s[g, 8*cc + 2*t + r]
        s_perm = np.ascontiguousarray(
            s_c.reshape(G, NP, 4, 2).transpose(0, 2, 1, 3).reshape(G, N_LOC)
        ).astype(ml_dtypes.bfloat16)
        # replica-major: [SREP, G, N_LOC]
        s_rep = np.ascontiguousarray(
            np.broadcast_to(s_perm[None, :, :], (SREP, G, N_LOC))
        )
        in_maps.append(
            {"xt": x_t, "qw": qw_pm, "sp": s_rep, "qz": qz_c, "oh": oh}
        )

    trace = os.environ.get("AWQ_KERNEL_TRACE", "0") == "1"
    if trace:
        _install_ntff_shim()

    nc = _get_nc()
    res = run_bass_kernel_spmd(
        nc, in_maps, core_ids=list(range(NCORES)), trace=trace
    )
    LAST_EXEC_NS = res.exec_time_ns
    if res.instructions_and_trace is not None:
        LAST_TRACE = res.instructions_and_trace[1]

    return np.concatenate(
        [np.asarray(res.results[i]["out"]).astype(np.float32)
         for i in range(NCORES)],
        axis=1,
    )
